# revision 22
# baseline (speedup 1.0000x reference)
"""Two-layer GAT (gnn_message_passing) on 8 Trainium2 NeuronCores.

Sharding: nodes are split into 8 contiguous shards of 1024 (one per core);
each core owns every edge whose destination is in its shard.  The halo
exchange is an on-device AllGather (Shared-output RDH path) of the per-node
feature table [h | alpha_src | alpha_dst]; cores then resolve incident edges
with batched indirect-gather DMAs.

Environment cost model (measured): ~40-60us fixed per instruction, and the
non-Shared-output collective path is ~100x slower than the Shared RDH path.
The kernel is therefore shaped to minimize instruction count and collective
bytes:
  - alpha_src/alpha_dst are folded into the projection weights host-side
    (W1ext = [W1.T | W1.T@bd(att_src) | W1.T@bd(att_dst)]), so the layer-1
    node table is produced purely by matmuls.
  - the final output only needs per-bond score sums, which are linear in h2,
    so layer 2 collapses to 15 functionals of x1 (per-head channel sums +
    alpha terms): table 2 is [8192, 16] instead of [8192, 192].
  - tables are fp16; both AllGathers write Shared outputs; the final bond
    scores use a 256B AllReduce instead of AllGathering x2.
  - message passing is destination-major with degree-sorted node blocks and
    one batched indirect gather per slot-chunk.
"""

import sys

if "/opt/trn_rl_repo" not in sys.path:
    sys.path.insert(0, "/opt/trn_rl_repo")

import numpy as np

import concourse.bacc as bacc
import concourse.mybir as mybir
import concourse.tile as tile
from concourse.bass import IndirectOffsetOnAxis
from concourse.bass_utils import run_bass_kernel_spmd
from concourse.masks import make_identity

F32 = mybir.dt.float32
FP16 = mybir.dt.float16
I32 = mybir.dt.int32
AF = mybir.ActivationFunctionType
OP = mybir.AluOpType
AX = mybir.AxisListType

N_NODES, N_EDGES = 8192, 49152
IN_F, HID, H1, H2, OUT_F = 128, 64, 64, 5, 32
N_BONDS = 64
N_CORES = 8
NC_SHARD = N_NODES // N_CORES      # 1024 nodes per core
P = 128
NB = NC_SHARD // P                 # 8 dst blocks per core
F1 = H1 * HID                      # 4096
T1W = F1 + 2 * H1                  # 4224: [h | alpha_src | alpha_dst]
T2W = 16                           # [hsum 5 | as2 5 | ad2 5 | pad]
KCH = 7                            # layer-1 slot chunk (SBUF-limited)
NEG = -1.0e30
NK = F1 // P                       # 32 k-chunks for the layer-2 projection


# ---------------------------------------------------------------- host side
def _prep(edge_index: np.ndarray):
    """Degree-sorted shard permutation + per-block padded slot tables."""
    src = np.concatenate([edge_index[0].astype(np.int64),
                          np.arange(N_NODES, dtype=np.int64)])
    dst = np.concatenate([edge_index[1].astype(np.int64),
                          np.arange(N_NODES, dtype=np.int64)])
    deg = np.bincount(dst, minlength=N_NODES)          # includes self loop
    # permute within each shard: degree descending (stable for determinism)
    newpos = np.empty(N_NODES, np.int64)               # node -> permuted global
    for c in range(N_CORES):
        lo = c * NC_SHARD
        order = np.argsort(-deg[lo:lo + NC_SHARD], kind="stable")
        newpos[lo + order] = lo + np.arange(NC_SHARD)
    # per-block slot counts (same for all cores at each block position)
    degp = np.empty(N_NODES, np.int64)
    degp[newpos] = deg
    kb = np.zeros(NB, np.int64)
    for c in range(N_CORES):
        lo = c * NC_SHARD
        blkmax = degp[lo:lo + NC_SHARD].reshape(NB, P).max(axis=1)
        kb = np.maximum(kb, blkmax)
    ks = tuple(int(v) for v in kb)
    tot = int(sum(ks))
    # slot tables: srcidx[core][128, tot]; padded slots point at row 0 and
    # are disabled by an additive -1e30 logit mask.
    srcidx = np.zeros((N_CORES, P, tot), np.int32)
    maskf = np.full((N_CORES, P, tot), NEG, np.float32)
    col0 = np.cumsum([0] + list(ks))[:-1]              # first col of block b
    # self loops at slot 0
    for c in range(N_CORES):
        lo = c * NC_SHARD
        rows = np.arange(NC_SHARD) % P
        cols = col0[np.arange(NC_SHARD) // P]
        srcidx[c, rows, cols] = lo + np.arange(NC_SHARD)
        maskf[c, rows, cols] = 0.0
    # real edges (self loops already placed at slot 0)
    fill = np.ones(N_NODES, np.int64)                  # next free slot per dst
    ps = newpos[src[:N_EDGES]]
    pdst = newpos[dst[:N_EDGES]]
    order = np.argsort(pdst, kind="stable")
    ps, pdst = ps[order], pdst[order]
    for s, d in zip(ps.tolist(), pdst.tolist()):
        c, loc = d // NC_SHARD, d % NC_SHARD
        k = fill[d]
        fill[d] += 1
        srcidx[c, loc % P, col0[loc // P] + k] = s
        maskf[c, loc % P, col0[loc // P] + k] = 0.0
    return ks, srcidx, maskf, newpos


def _make_core_inputs(inputs, prep, c):
    ks, srcidx, maskf, newpos = prep
    x = np.asarray(inputs["x"], np.float32)
    W1 = np.asarray(inputs["W1"], np.float32)
    W2 = np.asarray(inputs["W2"], np.float32)
    as1 = np.asarray(inputs["att_src1"], np.float32)
    ad1 = np.asarray(inputs["att_dst1"], np.float32)
    as2 = np.asarray(inputs["att_src2"], np.float32)
    ad2 = np.asarray(inputs["att_dst2"], np.float32)
    # permuted shard of x, transposed
    lo = c * NC_SHARD
    sel = np.empty(NC_SHARD, np.int64)                 # permuted pos -> node
    sel[newpos[lo:lo + NC_SHARD] - lo] = np.arange(lo, lo + NC_SHARD)
    # alpha contractions folded into the projections
    W1r = W1.reshape(H1, HID, IN_F)
    A1s = np.einsum("hci,hc->ih", W1r, as1)            # [128, 64]
    A1d = np.einsum("hci,hc->ih", W1r, ad1)
    W1ext = np.concatenate([W1.T, A1s, A1d], axis=1)   # [128, 4224]
    W2r = W2.reshape(H2, OUT_F, F1)
    Usum = W2r.sum(axis=1).T / H2                      # [4096, 5]
    A2s = np.einsum("hcj,hc->jh", W2r, as2)            # [4096, 5]
    A2d = np.einsum("hcj,hc->jh", W2r, ad2)
    U = np.concatenate([Usum, A2s, A2d,
                        np.zeros((F1, 1), np.float32)], axis=1)  # [4096, 16]
    Ub = U.reshape(NK, P, T2W).transpose(1, 0, 2).reshape(P, NK * T2W)
    # finale: local indices + ownership masks for bond endpoints
    pl = newpos[np.asarray(inputs["lefts"], np.int64)]
    pr = np.asarray(newpos[np.asarray(inputs["rights"], np.int64)])
    lown = (pl // NC_SHARD) == c
    rown = (pr // NC_SHARD) == c
    return {
        "xT": np.ascontiguousarray(x[sel].T).astype(np.float16),
        "W1ext": W1ext.astype(np.float16),
        "Ub": Ub.astype(np.float16),
        "srcidx": np.ascontiguousarray(srcidx[c]),
        "maskf": np.ascontiguousarray(maskf[c]),
        "lidx": np.where(lown, pl % NC_SHARD, 0)
            .astype(np.int32).reshape(N_BONDS, 1),
        "ridx": np.where(rown, pr % NC_SHARD, 0)
            .astype(np.int32).reshape(N_BONDS, 1),
        "lmask": lown.astype(np.float32).reshape(N_BONDS, 1),
        "rmask": rown.astype(np.float32).reshape(N_BONDS, 1),
    }


# ------------------------------------------------------------- device side
def _build_program(ks, reps: int = 1, skip=()) -> bacc.Bacc:
    nc = bacc.Bacc("TRN2", target_bir_lowering=False, debug=False,
                   num_devices=N_CORES)
    tot = int(sum(ks))
    col0 = np.cumsum([0] + list(ks))[:-1]
    kmax = max(ks)

    xT = nc.dram_tensor("xT", [P, NC_SHARD], FP16, kind="ExternalInput")
    W1ext = nc.dram_tensor("W1ext", [P, T1W], FP16, kind="ExternalInput")
    Ub = nc.dram_tensor("Ub", [P, NK * T2W], FP16, kind="ExternalInput")
    srcidx = nc.dram_tensor("srcidx", [P, tot], I32, kind="ExternalInput")
    maskf = nc.dram_tensor("maskf", [P, tot], F32, kind="ExternalInput")
    lidx = nc.dram_tensor("lidx", [N_BONDS, 1], I32, kind="ExternalInput")
    ridx = nc.dram_tensor("ridx", [N_BONDS, 1], I32, kind="ExternalInput")
    lmask = nc.dram_tensor("lmask", [N_BONDS, 1], F32, kind="ExternalInput")
    rmask = nc.dram_tensor("rmask", [N_BONDS, 1], F32, kind="ExternalInput")
    y = nc.dram_tensor("y", [N_BONDS], F32, kind="ExternalOutput")

    rg = [list(range(N_CORES))]

    with tile.TileContext(nc, num_cores=N_CORES) as tc:
        with (
            tc.tile_pool(name="dram", bufs=1, space="DRAM") as dpool,
            tc.tile_pool(name="consts", bufs=1) as cpool,
            tc.tile_pool(name="small", bufs=6) as mpool,
            tc.tile_pool(name="ps", bufs=1, space="PSUM") as ppool,
        ):
            srcidx_s = cpool.tile([P, tot], I32)
            maskf_s = cpool.tile([P, tot], F32)
            Ub_s = cpool.tile([P, NK * T2W], FP16)
            lidx_s = cpool.tile([N_BONDS, 1], I32)
            ridx_s = cpool.tile([N_BONDS, 1], I32)
            lmask_s = cpool.tile([N_BONDS, 1], F32)
            rmask_s = cpool.tile([N_BONDS, 1], F32)
            ident_s = cpool.tile([P, P], F32)
            for dt_, st_ in [(srcidx_s, srcidx), (maskf_s, maskf),
                             (Ub_s, Ub), (lidx_s, lidx), (ridx_s, ridx),
                             (lmask_s, lmask), (rmask_s, rmask)]:
                nc.sync.dma_start(dt_[:], st_[:])
            make_identity(nc, ident_s[:])

            for _rep in range(reps):
                t1_loc = dpool.tile([NC_SHARD, T1W], FP16, tag="t1l")
                t1_full = dpool.tile([N_NODES, T1W], FP16,
                                     addr_space="Shared", tag="t1f")
                x1_h = dpool.tile([NC_SHARD, F1], FP16, tag="x1h")
                t2_loc = dpool.tile([NC_SHARD, T2W], FP16, tag="t2l")
                t2_full = dpool.tile([N_NODES, T2W], FP16,
                                     addr_space="Shared", tag="t2f")
                s_dram = dpool.tile([NC_SHARD, 1], F32, tag="sdr")
                sp_loc = dpool.tile([N_BONDS, 1], F32, tag="spl")
                sp_full = dpool.tile([N_BONDS, 1], F32,
                                     addr_space="Shared", tag="spf")

                # ---- phase A: t1 rows = x @ [W1.T | A1s | A1d]
                with (
                    tc.tile_pool(name="pha", bufs=1) as hpool,
                    tc.tile_pool(name="phaw", bufs=2) as wpool,
                ):
                    xT_s = hpool.tile([P, NC_SHARD], FP16)
                    w1e_s = hpool.tile([P, T1W], FP16)
                    nc.sync.dma_start(xT_s[:], xT[:])
                    nc.sync.dma_start(w1e_s[:], W1ext[:])
                    for nt in range(NB):
                        lhs = xT_s[:, nt * P:(nt + 1) * P]
                        ps_h = ppool.tile([P, F1 // 2], F32, tag="psA")
                        for j in range(4):
                            nc.tensor.matmul(
                                ps_h[:, j * 512:(j + 1) * 512],
                                lhsT=lhs,
                                rhs=w1e_s[:, j * 512:(j + 1) * 512],
                                start=True, stop=True)
                        ps_i = ppool.tile([P, F1 // 2], F32, tag="psB")
                        for j in range(4):
                            nc.tensor.matmul(
                                ps_i[:, j * 512:(j + 1) * 512],
                                lhsT=lhs,
                                rhs=w1e_s[:, 2048 + j * 512:
                                          2048 + (j + 1) * 512],
                                start=True, stop=True)
                        h_s = wpool.tile([P, T1W], FP16, tag="h")
                        nc.vector.tensor_copy(h_s[:, 0:F1 // 2], ps_h[:])
                        nc.vector.tensor_copy(
                            h_s[:, F1 // 2:F1], ps_i[:])
                        ps_a = ppool.tile([P, F1 // 2], F32, tag="psA")
                        nc.tensor.matmul(
                            ps_a[:, 0:2 * H1],
                            lhsT=lhs,
                            rhs=w1e_s[:, F1:T1W],
                            start=True, stop=True)
                        nc.vector.tensor_copy(
                            h_s[:, F1:T1W], ps_a[:, 0:2 * H1])
                        nc.sync.dma_start(
                            t1_loc[nt * P:(nt + 1) * P, :], h_s[:])

                if "coll" not in skip:
                    nc.gpsimd.collective_compute(
                        "AllGather", OP.bypass, ins=[t1_loc.opt()],
                        outs=[t1_full.opt()], replica_groups=rg)

                # ---- layer-1 message passing (dst-major) -> x1 (fp16)
                with (
                    tc.tile_pool(name="l1g", bufs=2) as gpool,
                    tc.tile_pool(name="l1w", bufs=2) as wpool,
                ):
                    for blk in range(NB):
                        kb = ks[blk]
                        c0 = int(col0[blk])
                        num = wpool.tile([P, F1], F32, tag="num", bufs=1)
                        den = mpool.tile([P, H1], F32, tag="small")
                        adst = mpool.tile([P, H1], FP16, tag="smh")
                        first = True
                        for ck in range(0, kb, KCH):
                            ke = min(kb, ck + KCH)
                            kw = ke - ck
                            g = gpool.tile([P, KCH * T1W], FP16, tag="g")
                            gv = g[:, 0:kw * T1W].rearrange(
                                "p (k w) -> p k w", w=T1W)
                            for k in range(ck, ke):
                                if "gather" in skip:
                                    nc.sync.dma_start(
                                        gv[:, k - ck, :], t1_full[0:P, :])
                                else:
                                    nc.gpsimd.indirect_dma_start(
                                        out=gv[:, k - ck, :],
                                        out_offset=None,
                                        in_=t1_full[:],
                                        in_offset=IndirectOffsetOnAxis(
                                            ap=srcidx_s[:, c0 + k:c0 + k + 1],
                                            axis=0))
                            if ck == 0 and kb > KCH:
                                nc.vector.tensor_copy(
                                    adst[:], gv[:, 0, F1 + H1:T1W])
                            adsrc = (adst[:] if kb > KCH
                                     else gv[:, 0, F1 + H1:T1W])
                            w = mpool.tile([P, KCH * H1], F32, tag="wsl")
                            wv = w[:, 0:kw * H1].rearrange(
                                "p (k h) -> p k h", h=H1)
                            nc.vector.tensor_tensor(
                                out=wv, in0=gv[:, :, F1:F1 + H1],
                                in1=adsrc.unsqueeze(1)
                                    .broadcast_to([P, kw, H1]),
                                op=OP.add)
                            nc.vector.tensor_tensor(
                                out=wv, in0=wv,
                                in1=maskf_s[:, c0 + ck:c0 + ke].unsqueeze(2)
                                    .broadcast_to([P, kw, H1]),
                                op=OP.add)
                            nc.vector.scalar_tensor_tensor(
                                out=wv, in0=wv, scalar=0.2, in1=wv,
                                op0=OP.mult, op1=OP.max)
                            wh = mpool.tile([P, KCH * H1], FP16, tag="wslh")
                            wvh = wh[:, 0:kw * H1].rearrange(
                                "p (k h) -> p k h", h=H1)
                            nc.scalar.activation(wvh, wv, AF.Exp)
                            nc.vector.tensor_tensor(
                                out=gv[:, :, 0:F1]
                                    .rearrange("p k (h c) -> p k h c", c=HID),
                                in0=gv[:, :, 0:F1]
                                    .rearrange("p k (h c) -> p k h c", c=HID),
                                in1=wvh.unsqueeze(3)
                                    .broadcast_to([P, kw, H1, HID]),
                                op=OP.mult)
                            if first:
                                nc.vector.tensor_reduce(
                                    out=num[:],
                                    in_=gv[:, :, 0:F1].transpose([0, 2, 1]),
                                    axis=AX.X, op=OP.add)
                                nc.vector.tensor_reduce(
                                    out=den[:], in_=wvh.transpose([0, 2, 1]),
                                    axis=AX.X, op=OP.add)
                            else:
                                pnum = wpool.tile([P, F1], F32, tag="zz",
                                                  bufs=1)
                                nc.vector.tensor_reduce(
                                    out=pnum[:],
                                    in_=gv[:, :, 0:F1].transpose([0, 2, 1]),
                                    axis=AX.X, op=OP.add)
                                nc.vector.tensor_tensor(
                                    out=num[:], in0=num[:], in1=pnum[:],
                                    op=OP.add)
                                pden = mpool.tile([P, H1], F32, tag="small")
                                nc.vector.tensor_reduce(
                                    out=pden[:], in_=wvh.transpose([0, 2, 1]),
                                    axis=AX.X, op=OP.add)
                                nc.vector.tensor_tensor(
                                    out=den[:], in0=den[:], in1=pden[:],
                                    op=OP.add)
                            first = False
                        # x1 = elu(num / den)   (b1 is zeros by problem spec)
                        dinv = mpool.tile([P, H1], F32, tag="small")
                        nc.vector.reciprocal(dinv[:], den[:])
                        z = wpool.tile([P, F1], FP16, tag="zz", bufs=1)
                        nc.vector.tensor_tensor(
                            out=z[:].rearrange("p (h c) -> p h c", c=HID),
                            in0=num[:].rearrange("p (h c) -> p h c", c=HID),
                            in1=dinv[:].unsqueeze(2)
                                .broadcast_to([P, H1, HID]),
                            op=OP.mult)
                        zm = wpool.tile([P, F1], FP16, tag="num", bufs=1)
                        nc.vector.tensor_scalar_min(zm[:], z[:], 0.0)
                        nc.scalar.activation(zm[:], zm[:], AF.Exp)
                        nc.scalar.activation(z[:], z[:], AF.Relu)
                        x1n = wpool.tile([P, F1], FP16, tag="x1n", bufs=1)
                        nc.vector.scalar_tensor_tensor(
                            out=x1n[:], in0=zm[:], scalar=-1.0, in1=z[:],
                            op0=OP.add, op1=OP.add)
                        nc.sync.dma_start(
                            x1_h[blk * P:(blk + 1) * P, :], x1n[:])

                # ---- layer-2 projection: t2 = x1 @ U (feature-major)
                with tc.tile_pool(name="prj", bufs=2) as jpool:
                    ps_u = ppool.tile([P, F1 // 2], F32, tag="psA")
                    for k in range(NK):
                        x1t = jpool.tile([P, NC_SHARD], FP16, tag="x1t")
                        nc.sync.dma_start(
                            x1t[:], x1_h[:, k * P:(k + 1) * P],
                            transpose=True)
                        for nh in range(2):
                            nc.tensor.matmul(
                                ps_u[0:T2W, nh * 512:(nh + 1) * 512],
                                lhsT=Ub_s[:, k * T2W:(k + 1) * T2W],
                                rhs=x1t[:, nh * 512:(nh + 1) * 512],
                                start=(k == 0), stop=(k == NK - 1),
                                skip_group_check=True)
                    t2f = jpool.tile([T2W, NC_SHARD], F32, tag="t2f",
                                     bufs=1)
                    nc.vector.tensor_copy(t2f[:], ps_u[0:T2W, 0:NC_SHARD])
                    # transpose back to node-major rows and write table 2
                    for nt in range(NB):
                        ps_t = ppool.tile([P, F1 // 2], F32, tag="psB")
                        nc.tensor.transpose(
                            out=ps_t[:, 0:T2W],
                            in_=t2f[:, nt * P:(nt + 1) * P],
                            identity=ident_s[0:T2W, 0:T2W])
                        row = jpool.tile([P, T2W], FP16, tag="row")
                        nc.vector.tensor_copy(row[:], ps_t[:, 0:T2W])
                        nc.sync.dma_start(
                            t2_loc[nt * P:(nt + 1) * P, :], row[:])

                if "coll" not in skip:
                    nc.gpsimd.collective_compute(
                        "AllGather", OP.bypass, ins=[t2_loc.opt()],
                        outs=[t2_full.opt()], replica_groups=rg)

                # ---- layer-2 message passing: all 8 blocks batched into
                # global ops over one [128, tot*16] tile (rows are tiny)
                with tc.tile_pool(name="l2g", bufs=1) as g2pool:
                    g2 = g2pool.tile([P, tot * T2W], FP16, tag="g2")
                    gv = g2[:].rearrange("p (k w) -> p k w", w=T2W)
                    for s in range(tot):
                        if "gather" in skip:
                            nc.sync.dma_start(gv[:, s, :], t2_full[0:P, :])
                        else:
                            nc.gpsimd.indirect_dma_start(
                                out=gv[:, s, :], out_offset=None,
                                in_=t2_full[:],
                                in_offset=IndirectOffsetOnAxis(
                                    ap=srcidx_s[:, s:s + 1], axis=0))
                    w2 = g2pool.tile([P, tot * H2], F32, tag="w2s")
                    wva = w2[:].rearrange("p (k h) -> p k h", h=H2)
                    for blk in range(NB):
                        kb = ks[blk]
                        c0 = int(col0[blk])
                        nc.vector.tensor_tensor(
                            out=wva[:, c0:c0 + kb, :],
                            in0=gv[:, c0:c0 + kb, H2:2 * H2],
                            in1=gv[:, c0, 2 * H2:3 * H2].unsqueeze(1)
                                .broadcast_to([P, kb, H2]),
                            op=OP.add)
                    nc.vector.tensor_tensor(
                        out=wva, in0=wva,
                        in1=maskf_s[:, 0:tot].unsqueeze(2)
                            .broadcast_to([P, tot, H2]),
                        op=OP.add)
                    nc.vector.scalar_tensor_tensor(
                        out=wva, in0=wva, scalar=0.2, in1=wva,
                        op0=OP.mult, op1=OP.max)
                    w2h = g2pool.tile([P, tot * H2], FP16, tag="w2h")
                    wvh = w2h[:].rearrange("p (k h) -> p k h", h=H2)
                    nc.scalar.activation(wvh, wva, AF.Exp)
                    nc.vector.tensor_tensor(
                        out=gv[:, :, 0:H2], in0=gv[:, :, 0:H2],
                        in1=wvh, op=OP.mult)
                    num_a = g2pool.tile([P, NB * H2], F32, tag="numa")
                    den_a = g2pool.tile([P, NB * H2], F32, tag="dena")
                    for blk in range(NB):
                        kb = ks[blk]
                        c0 = int(col0[blk])
                        nc.vector.tensor_reduce(
                            out=num_a[:, blk * H2:(blk + 1) * H2],
                            in_=gv[:, c0:c0 + kb, 0:H2].transpose([0, 2, 1]),
                            axis=AX.X, op=OP.add)
                        nc.vector.tensor_reduce(
                            out=den_a[:, blk * H2:(blk + 1) * H2],
                            in_=wvh[:, c0:c0 + kb, :].transpose([0, 2, 1]),
                            axis=AX.X, op=OP.add)
                    nc.vector.reciprocal(den_a[:], den_a[:])
                    nc.vector.tensor_tensor(
                        out=num_a[:], in0=num_a[:], in1=den_a[:],
                        op=OP.mult)
                    s_a = g2pool.tile([P, NB], F32, tag="sa")
                    nc.vector.tensor_reduce(
                        out=s_a[:],
                        in_=num_a[:].rearrange("p (b h) -> p b h", h=H2),
                        axis=AX.X, op=OP.add)
                    nc.sync.dma_start(
                        s_dram[:].rearrange("(b p) o -> p (b o)", p=P),
                        s_a[:])

                # ---- finale: bond scores, 256B AllReduce, softmax
                if "phD" not in skip:
                    gl = mpool.tile([N_BONDS, 1], F32, tag="small")
                    gr = mpool.tile([N_BONDS, 1], F32, tag="small")
                    nc.gpsimd.indirect_dma_start(
                        out=gl[:], out_offset=None, in_=s_dram[:],
                        in_offset=IndirectOffsetOnAxis(
                            ap=lidx_s[:, 0:1], axis=0))
                    nc.gpsimd.indirect_dma_start(
                        out=gr[:], out_offset=None, in_=s_dram[:],
                        in_offset=IndirectOffsetOnAxis(
                            ap=ridx_s[:, 0:1], axis=0))
                    nc.vector.tensor_tensor(
                        out=gl[:], in0=gl[:], in1=lmask_s[:], op=OP.mult)
                    nc.vector.tensor_tensor(
                        out=gr[:], in0=gr[:], in1=rmask_s[:], op=OP.mult)
                    nc.vector.tensor_tensor(
                        out=gl[:], in0=gl[:], in1=gr[:], op=OP.add)
                    nc.sync.dma_start(sp_loc[:], gl[:])
                    if "coll" not in skip:
                        nc.gpsimd.collective_compute(
                            "AllReduce", OP.add, ins=[sp_loc.opt()],
                            outs=[sp_full.opt()], replica_groups=rg)
                    sc = mpool.tile([N_BONDS, 1], F32, tag="small")
                    nc.sync.dma_start(sc[:], sp_full[:])
                    ps_sc = ppool.tile([P, F1 // 2], F32, tag="psA")
                    nc.tensor.transpose(
                        out=ps_sc[0:1, 0:N_BONDS], in_=sc[:],
                        identity=ident_s[0:N_BONDS, 0:N_BONDS])
                    es = mpool.tile([1, N_BONDS], F32, tag="small")
                    nc.scalar.activation(es[:], ps_sc[0:1, 0:N_BONDS], AF.Exp)
                    ssum = mpool.tile([1, 1], F32, tag="small")
                    nc.vector.tensor_reduce(
                        out=ssum[:], in_=es[:], axis=AX.X, op=OP.add)
                    sinv = mpool.tile([1, 1], F32, tag="small")
                    nc.vector.reciprocal(sinv[:], ssum[:])
                    ys = mpool.tile([1, N_BONDS], F32, tag="small")
                    nc.vector.tensor_tensor(
                        out=ys[:], in0=es[:],
                        in1=sinv[:].to_broadcast([1, N_BONDS]), op=OP.mult)
                    nc.sync.dma_start(y.ap().unsqueeze(0), ys[:])

    nc.compile()
    return nc


_PROGRAM_CACHE: dict = {}


def kernel(**inputs) -> np.ndarray:
    prep = _prep(np.asarray(inputs["edge_index"], np.int64))
    ks = prep[0]
    if ks not in _PROGRAM_CACHE:
        _PROGRAM_CACHE[ks] = _build_program(ks)
    nc = _PROGRAM_CACHE[ks]
    in_maps = [_make_core_inputs(inputs, prep, c) for c in range(N_CORES)]
    res = run_bass_kernel_spmd(nc, in_maps, core_ids=list(range(N_CORES)))
    return res.results[0]["y"]


if __name__ == "__main__":
    import jax

    import reference

    with jax.default_device(jax.devices("cpu")[0]):
        inputs = {k: np.asarray(v) for k, v in reference.setup_inputs().items()}
        expected = np.asarray(reference.reference(**reference.setup_inputs()))
    actual = kernel(**inputs)
    rel = np.abs(actual - expected).max() / np.abs(expected).max()
    print("Relative error:", rel)
